# revision 25
# baseline (speedup 1.0000x reference)
"""Causal multi-head attention block on 8 NeuronCores (Trainium2, Bass/Tile).

Reference computation (per batch b):
  Q = x @ W_Q + b_Q ; K = x @ W_K + b_K ; V = x @ W_V + b_V   (per head)
  scores = Q K^T / sqrt(H); causal mask; probs = softmax(scores)
  out = (probs @ V) @ W_O + b_O

Sharding: core c -> batch c//2, head-group c%2 (6 of 12 heads).
Each core computes a partial output [S, D] (its heads' contribution,
with b_Q/b_K applied on-device). Host sums the two head-group partials
per batch and adds b_O + sum_nh b_V[n,h] * W_O[n,h,:] (exact: the b_V
term factors out because softmax rows sum to 1).

Device-side layout choices (v3, bf16 + interleaved schedule):
  - all matmul operands are bf16 (same 1 cycle/row PE rate as fp32r but
    with no >=256 moving-dim constraint); PSUM accumulation stays fp32.
    DMA traffic halves; output is written bf16 and upcast on host.
  - scores are computed transposed ([k, q]); the softmax sum over k is
    taken by the PV matmul via a ones column on V.
  - the two heads of a pair share one 2-bank PSUM score tile
    [128, 2, 512]; a single Exp activation covers both heads. Score
    tiles are double-buffered so the PE can run a k-tile ahead of Exp.
  - the attention inner loop is Activation-paced (exp ~0.9us/k-tile vs
    ~0.64us of PE work), so PE filler work (s2=1 projections, qb0
    output-projection groups) is interleaved between k-tiles to keep
    the PE busy through the attention phases.
  - qb1 of the last pair streams its softmax normalize per 256-column
    half (columns [0:256] are final after k-tile 5), so the final
    output-projection tiles overlap the tail of attention instead of
    serializing after it.
  - engine balance: exp + qb0 bias-adds + dh0 out-copies on Activation;
    reciprocal/normalize + s2=1 bias-adds + dh1 out-copies on DVE;
    causal masks, V-copies, and broadcasts on gpsimd.
  - projections, warm-up, and output-projection accumulators share one
    double-buffered 1-bank PSUM pool (8 banks total in use).
"""

import sys

sys.path.insert(0, "/opt/trn_rl_repo")

from contextlib import ExitStack

import ml_dtypes
import numpy as np

import concourse.bass as bass
import concourse.tile as tile
from concourse import bacc, mybir
from concourse.bass_utils import run_bass_kernel_spmd

B, S, D, N, H = 4, 1024, 768, 12, 64
NHC = 6            # heads per core
NPAIR = NHC // 2   # head pairs per core (2 heads stacked -> 128 partitions)
HD = NHC * H       # 384: per-core packed head dim
P = 128
NDT = D // P       # 6 d-tiles
NST = S // P       # 8 s-tiles (also k-tiles)
QB = 512           # q block (moving-dim tile for most matmuls)
NQB = S // QB      # 2
F32 = mybir.dt.float32
BF16 = mybir.dt.bfloat16
FP8 = mybir.dt.float8e4
NDC = D // 256     # 3 DoubleRow d-chunks (256 contraction rows each)
W8_SCALE = 16.0    # host-side W_Q/W_K scale keeping fp8 out of subnormals
EXP_SCALE = 1.0 / np.sqrt(float(H))

_CACHE = {}


def _build():
    nc = bacc.Bacc()
    xt8_d = nc.declare_dram_parameter("xt8", [D, S], FP8, isOutput=False)
    xtr_d = nc.declare_dram_parameter("xtr8", [D, S], FP8, isOutput=False)
    wq_d = nc.declare_dram_parameter("wq8", [D, HD], FP8, isOutput=False)
    wk_d = nc.declare_dram_parameter("wk8", [D, HD], FP8, isOutput=False)
    wv_d = nc.declare_dram_parameter("wv8", [D, HD], FP8, isOutput=False)
    wvr_d = nc.declare_dram_parameter("wvr8", [D, HD], FP8, isOutput=False)
    wo_d = nc.declare_dram_parameter("wo", [HD, D], BF16, isOutput=False)
    bq_d = nc.declare_dram_parameter("bq", [P, NPAIR], F32, isOutput=False)
    bk_d = nc.declare_dram_parameter("bk", [P, NPAIR], F32, isOutput=False)
    tri_d = nc.declare_dram_parameter("trimask", [P, 2 * P], BF16, isOutput=False)
    out_d = nc.declare_dram_parameter("out", [S, D], BF16, isOutput=True)

    # fp8 path: DoubleRow matmuls contract 2 rows per partition, so x and
    # the W matrices are addressed as [p, chunk, i, *], d = c*256 + i*128 + p.
    # xr8/wvr8 are fp8 quantization residuals: adding their cross-terms into
    # the same PSUM group (same dequant scale) cancels most of the fp8 error.
    xt8_r = xt8_d[:].rearrange("(c i p) s -> p c i s", p=P, i=2)
    xtr_r = xtr_d[:].rearrange("(c i p) s -> p c i s", p=P, i=2)
    wq_r = wq_d[:].rearrange("(c i p) h -> p c i h", p=P, i=2)
    wk_r = wk_d[:].rearrange("(c i p) h -> p c i h", p=P, i=2)
    wv_r = wv_d[:].rearrange("(c i p) h -> p c i h", p=P, i=2)
    wvr_r = wvr_d[:].rearrange("(c i p) h -> p c i h", p=P, i=2)
    wo_r = wo_d[:].rearrange("(t p) d -> p t d", p=P)

    with tile.TileContext(nc) as tc, ExitStack() as ctx:
        consts = ctx.enter_context(tc.tile_pool(name="consts", bufs=1))
        persist = ctx.enter_context(tc.tile_pool(name="persist", bufs=1))
        etp = ctx.enter_context(tc.tile_pool(name="etp", bufs=4))
        smalls = ctx.enter_context(tc.tile_pool(name="smalls", bufs=4))
        outp = ctx.enter_context(tc.tile_pool(name="outp", bufs=3))

        x8 = consts.tile([P, NDC, 2, S], FP8)
        xr8 = consts.tile([P, NDC, 2, S], FP8)
        wq_sb = consts.tile([P, NDC, 2, HD], FP8)
        wk_sb = consts.tile([P, NDC, 2, HD], FP8)
        wv_sb = consts.tile([P, NDC, 2, HD], FP8)
        wvr_sb = consts.tile([P, NDC, 2, HD], FP8)
        bq_sb = consts.tile([P, NPAIR], F32)
        bk_sb = consts.tile([P, NPAIR], F32)
        tri = consts.tile([P, 2, P], BF16)
        wo_sb = consts.tile([P, NPAIR, D], BF16)

        # ---- DMA emission order == priority order on the shared DMA device.
        # Q/K projections of all pairs run first (need only wq8/wk8 + x8 qb0),
        # then V; later phases' tensors stream behind.
        nc.sync.dma_start(out=wq_sb, in_=wq_r)
        nc.sync.dma_start(out=x8[:, :, :, 0:QB], in_=xt8_r[:, :, :, 0:QB])
        nc.sync.dma_start(out=bq_sb, in_=bq_d[:])
        nc.sync.dma_start(out=wk_sb, in_=wk_r)
        nc.sync.dma_start(out=bk_sb, in_=bk_d[:])
        nc.sync.dma_start(out=xr8[:, :, :, 0:QB], in_=xtr_r[:, :, :, 0:QB])
        nc.sync.dma_start(out=wv_sb, in_=wv_r)
        nc.sync.dma_start(out=wvr_sb, in_=wvr_r)
        nc.sync.dma_start(
            out=tri, in_=tri_d[:].rearrange("p (two q) -> p two q", two=2)
        )
        nc.sync.dma_start(out=x8[:, :, :, QB:S], in_=xt8_r[:, :, :, QB:S])
        nc.sync.dma_start(out=xr8[:, :, :, QB:S], in_=xtr_r[:, :, :, QB:S])
        nc.sync.dma_start(out=wo_sb, in_=wo_r)

        # ---- persistent activations ----
        qT = persist.tile([P, NPAIR, S], BF16)     # Q^T, head pairs stacked
        kT = persist.tile([P, NPAIR, S], BF16)
        vA = persist.tile([P, NST, NHC, H + 1], BF16)  # V + ones col, per k-tile
        zT = persist.tile([P, NPAIR, S], BF16)     # z^T (normalized), pairs stacked

        nc.gpsimd.memset(vA[:, :, :, H : H + 1], 1.0)

        # Shared 1-bank accumulator pool: warm-up, Q/K/V projections, and
        # output-projection groups all round-robin its two buffers.
        ps_big = ctx.enter_context(tc.tile_pool(name="ps_big", bufs=2, space="PSUM"))
        # Score tiles: [128, 2, 512] fp32 = 2 banks each, double-buffered.
        ps_s = ctx.enter_context(tc.tile_pool(name="ps_s", bufs=2, space="PSUM"))
        # z accumulators (one per head of the active pair): 1 bank each.
        ps_z = ctx.enter_context(tc.tile_pool(name="ps_z", bufs=1, space="PSUM"))

        # PE warm-up: matmuls on a zeroed tile depend on no DMA, so they run
        # during the input-stream prologue and carry the PE clock (HAM) and
        # cost-model p-state ramp to full speed before the first real matmul.
        dums = consts.tile([P, QB], BF16)
        nc.gpsimd.memset(dums, 0.0)
        # Activation-table preload: the first table-based activation pays a
        # 1283ns ACT_TABLE_LOAD; trigger it at t=0 on a dummy tile so the
        # first bias-add (which gates the ps_big ring) doesn't.
        actwarm = consts.tile([1, 1], F32)
        nc.gpsimd.memset(actwarm, 0.0)
        nc.scalar.activation(
            actwarm, actwarm, mybir.ActivationFunctionType.Exp
        )
        wps = ps_big.tile([P, QB], F32, name="warm", tag="big")
        for i in range(12):
            nc.tensor.matmul(
                wps,
                dums[:, 0:P],
                dums,
                start=(i == 0),
                stop=(i == 11),
            )

        def proj_one(w_sb, b_sb, dst, g, s2, eng):
            """Q/K projection of one head pair over one q-half: fp8 DoubleRow
            matmuls (256 contraction rows per instruction, 0.5 cycles/row),
            two sets (x8 + its fp8 residual) accumulated in one PSUM group.
            The bias-add applies the 1/W8_SCALE dequant."""
            ps = ps_big.tile([P, QB], F32, tag="big")
            for si, xs in enumerate((x8, xr8)):
                for c in range(NDC):
                    nc.tensor.matmul(
                        ps,
                        w_sb[:, c, :, g * P : (g + 1) * P],
                        xs[:, c, :, s2 * QB : (s2 + 1) * QB],
                        start=(si == 0 and c == 0),
                        stop=(si == 1 and c == NDC - 1),
                        perf_mode=mybir.MatmulPerfMode.DoubleRow,
                    )
            dst_ap = dst[:, g, s2 * QB : (s2 + 1) * QB]
            if eng == "act":
                nc.scalar.activation(
                    dst_ap,
                    ps,
                    mybir.ActivationFunctionType.Identity,
                    bias=b_sb[:, g : g + 1],
                    scale=1.0 / W8_SCALE,
                )
            else:
                nc.vector.tensor_scalar(
                    dst_ap,
                    ps,
                    1.0 / W8_SCALE,
                    b_sb[:, g : g + 1],
                    mybir.AluOpType.mult,
                    mybir.AluOpType.add,
                )

        def proj_v(st):
            """V projection of one k-tile: three fp8 DoubleRow sets
            (x8*wv8 + xr8*wv8 + x8*wvr8 — both residual cross-terms, same
            dequant scale) so V carries ~0.2% error despite fp8 operands."""
            vps = ps_big.tile([P, HD], F32, tag="big")
            sets = ((x8, wv_sb), (xr8, wv_sb), (x8, wvr_sb))
            for si, (xs, ws) in enumerate(sets):
                for c in range(NDC):
                    nc.tensor.matmul(
                        vps,
                        xs[:, c, :, st * P : (st + 1) * P],
                        ws[:, c, :, :],
                        start=(si == 0 and c == 0),
                        stop=(si == 2 and c == NDC - 1),
                        perf_mode=mybir.MatmulPerfMode.DoubleRow,
                    )
            nc.vector.tensor_scalar_mul(
                vA[:, st, :, 0:H],
                vps.rearrange("p (n h) -> p n h", n=NHC),
                1.0 / W8_SCALE,
            )

        def norm_block(zzps, g, q0, c0, c1):
            """Normalize z columns [c0, c1) of pair g's block at q offset q0."""
            w = c1 - c0
            for hh in range(2):
                hp = hh * H
                r = smalls.tile([1, w], F32, tag="r")
                nc.vector.reciprocal(r, zzps[hh][H : H + 1, c0:c1])
                rb = smalls.tile([H, w], F32, tag="rb")
                nc.gpsimd.partition_broadcast(rb, r)
                nc.vector.tensor_mul(
                    zT[hp : hp + H, g, q0 + c0 : q0 + c1],
                    zzps[hh][0:H, c0:c1],
                    rb,
                )

        def attend_pair(g, qb, fillers=None, post_kt=None):
            """Both heads of pair g: one 2-bank score tile per k-tile, one
            merged Exp per k-tile; PV accumulates z^T + softmax denominator
            via the ones column of vA. `fillers` are PE work units popped
            one per k-tile to cover the Exp-paced stretches; `post_kt` maps
            k-tile index -> closures run right after that k-tile's PV (used
            to stream the last pair's normalize + output tiles)."""
            q0 = qb * QB
            nkt = (qb + 1) * QB // P  # causal: k-tiles 0..nkt-1
            zzps = [
                ps_z.tile([H + 1, QB], F32, name=f"zps{hh}", tag=f"z{hh}")
                for hh in range(2)
            ]
            # One filler ahead of k-tile 0: its PV may wait on the previous
            # pair's normalize to release the z banks.
            if fillers:
                fillers.pop(0)()
            for kt in range(nkt):
                o = max(kt * P - q0, 0)  # first live column
                sps = ps_s.tile([P, 2, QB], F32, tag="s")
                for hh in range(2):
                    hp = hh * H
                    nc.tensor.matmul(
                        sps[:, hh, o:QB],
                        kT[hp : hp + H, g, kt * P : (kt + 1) * P],
                        qT[hp : hp + H, g, q0 + o : q0 + QB],
                        start=True,
                        stop=True,
                        tile_position=(hp, 0),
                    )
                et = etp.tile([P, 2, QB], BF16)
                nc.scalar.activation(
                    et[:, :, o:QB],
                    sps[:, :, o:QB],
                    mybir.ActivationFunctionType.Exp,
                    scale=EXP_SCALE,
                )
                if kt * P - q0 >= -(P - 1):  # diagonal tile: partial block
                    nc.gpsimd.tensor_mul(
                        et[:, :, o : o + P], et[:, :, o : o + P], tri
                    )
                for hh in range(2):
                    nc.tensor.matmul(
                        zzps[hh][:, o:QB],
                        vA[:, kt, 2 * g + hh, :],
                        et[:, hh, o:QB],
                        start=(kt == 0),
                        stop=(kt == nkt - 1),
                    )
                if post_kt and kt in post_kt:
                    for fn in post_kt[kt]:
                        fn(zzps)
                if fillers:
                    fillers.pop(0)()
            return zzps

        def out_group(row0, dh, out_t, copy_eng="dve", ops=None):
            """One output-projection accumulation group: rows [row0, row0+P),
            column half dh."""
            if ops is None:
                ops = ps_big.tile([P, D // 2], F32, tag="big")
            for g in range(NPAIR):
                nc.tensor.matmul(
                    ops,
                    zT[:, g, row0 : row0 + P],
                    wo_sb[:, g, dh * (D // 2) : (dh + 1) * (D // 2)],
                    start=(g == 0),
                    stop=(g == NPAIR - 1),
                )
            dst = out_t[:, dh * (D // 2) : (dh + 1) * (D // 2)]
            if copy_eng == "act":
                nc.scalar.copy(dst, ops)
            else:
                nc.vector.tensor_copy(out=dst, in_=ops)

        def out_tile(row0, tail=False):
            """Full output tile rows [row0, row0+P): both dh groups, one DMA.
            Tiles emitted after the last Exp borrow the (drained) score pool
            for their accumulators and split copies across Act/DVE, so the
            serial tail is just mm -> copy -> DMA."""
            out_t = outp.tile([P, D], BF16)
            if tail:
                ops2 = ps_s.tile([P, 2, QB], F32, tag="s")
                out_group(row0, 0, out_t, copy_eng="act", ops=ops2[:, 0, 0 : D // 2])
                out_group(row0, 1, out_t, ops=ops2[:, 1, 0 : D // 2])
            else:
                out_group(row0, 0, out_t)
                out_group(row0, 1, out_t)
            nc.sync.dma_start(out=out_d[row0 : row0 + P, :], in_=out_t)

        # ---- schedule: all qb0 Q/K projections first (cheap fp8 DoubleRow,
        # need only wq8/wk8 + x8 qb0); V projections and qb0 output tiles
        # ride as fillers inside the Activation-paced attention loops so the
        # PE never idles on Exp.
        for g in range(NPAIR):
            proj_one(wq_sb, bq_sb, qT, g, 0, "act")
        proj_one(wk_sb, bk_sb, kT, 0, 0, "act")
        proj_one(wk_sb, bk_sb, kT, 1, 0, "dve")
        proj_one(wk_sb, bk_sb, kT, 2, 0, "dve")
        proj_v(0)
        z = attend_pair(0, 0, fillers=[
            lambda: proj_v(1),
            lambda: proj_v(2),
            lambda: proj_v(3),
        ])
        norm_block(z, 0, 0, 0, QB)
        z = attend_pair(1, 0, fillers=[
            lambda: proj_one(wq_sb, bq_sb, qT, 0, 1, "dve"),
            lambda: proj_one(wk_sb, bk_sb, kT, 0, 1, "dve"),
        ])
        norm_block(z, 1, 0, 0, QB)
        z = attend_pair(2, 0, fillers=[
            lambda: proj_v(4),
            lambda: proj_v(5),
            lambda: proj_v(6),
            lambda: proj_v(7),
        ])
        norm_block(z, 2, 0, 0, QB)

        z = attend_pair(0, 1, fillers=[
            lambda: proj_one(wq_sb, bq_sb, qT, 1, 1, "dve"),
            lambda: proj_one(wk_sb, bk_sb, kT, 1, 1, "dve"),
            lambda: out_tile(0 * P),
            lambda: proj_one(wq_sb, bq_sb, qT, 2, 1, "dve"),
            lambda: proj_one(wk_sb, bk_sb, kT, 2, 1, "dve"),
            lambda: out_tile(1 * P),
        ])
        norm_block(z, 0, QB, 0, QB)

        f11 = [lambda qt=qt: out_tile(qt * P) for qt in range(2, 4)]
        z = attend_pair(1, 1, fillers=f11)
        norm_block(z, 1, QB, 0, QB)

        # Last pair: stream the normalize per 128-column block (block qt of
        # qb1 is final after k-tile 4+qt), so qb1 output tiles overlap the
        # tail of attention; norms are emitted before out-tiles so the DVE
        # starts each block's chain while the PE runs the out-tile matmuls.
        post = {
            4: [lambda zz: norm_block(zz, 2, QB, 0, P)],
            5: [
                lambda zz: norm_block(zz, 2, QB, P, 2 * P),
                lambda zz: out_tile(QB + 0 * P),
            ],
            6: [
                lambda zz: norm_block(zz, 2, QB, 2 * P, 3 * P),
                lambda zz: out_tile(QB + 1 * P),
            ],
            7: [
                lambda zz: norm_block(zz, 2, QB, 3 * P, QB),
                lambda zz: out_tile(QB + 2 * P, tail=True),
            ],
        }
        z = attend_pair(2, 1, post_kt=post)
        out_tile(QB + 3 * P, tail=True)

    if not nc.is_finalized():
        nc.finalize()
    return nc


def _get_program():
    if "nc" not in _CACHE:
        _CACHE["nc"] = _build()
    return _CACHE["nc"]


def make_in_maps(
    normalized_resid_pre, W_Q, W_K, W_V, W_O, b_Q, b_K, b_V=None, b_O=None, **_unused
):
    bf = ml_dtypes.bfloat16
    f8 = ml_dtypes.float8_e4m3
    x = np.asarray(normalized_resid_pre, np.float32)
    W_Q, W_K, W_V = (np.asarray(a, np.float32) for a in (W_Q, W_K, W_V))
    W_O = np.asarray(W_O, np.float32)
    b_Q, b_K = np.asarray(b_Q, np.float32), np.asarray(b_K, np.float32)

    tri = np.triu(np.ones((P, P), np.float32))
    tri2 = np.concatenate([tri, tri], axis=1).astype(bf)
    in_maps = []
    for c in range(8):
        b, hg = divmod(c, 2)
        hs = slice(hg * NHC, (hg + 1) * NHC)
        xt = x[b].T
        xt8 = xt.astype(f8)
        wq8 = (W_Q[hs] * W8_SCALE).transpose(1, 0, 2).reshape(D, HD).astype(f8)
        wk8 = (W_K[hs] * W8_SCALE).transpose(1, 0, 2).reshape(D, HD).astype(f8)
        wvs = (W_V[hs] * W8_SCALE).transpose(1, 0, 2).reshape(D, HD)
        wv8 = wvs.astype(f8)
        in_maps.append(
            {
                "xt8": np.ascontiguousarray(xt8),
                "xtr8": np.ascontiguousarray(
                    (xt - xt8.astype(np.float32)).astype(f8)
                ),
                "wq8": np.ascontiguousarray(wq8),
                "wk8": np.ascontiguousarray(wk8),
                "wv8": np.ascontiguousarray(wv8),
                "wvr8": np.ascontiguousarray(
                    (wvs - wv8.astype(np.float32)).astype(f8)
                ),
                "wo": np.ascontiguousarray(W_O[hs].reshape(HD, D).astype(bf)),
                "bq": np.ascontiguousarray(b_Q[hs].reshape(NPAIR, P).T),
                "bk": np.ascontiguousarray(b_K[hs].reshape(NPAIR, P).T),
                "trimask": tri2,
            }
        )
    return in_maps


def kernel(
    normalized_resid_pre, W_Q, W_K, W_V, W_O, b_Q, b_K, b_V, b_O, **_unused
):
    W_O = np.asarray(W_O, np.float32)
    b_V, b_O = np.asarray(b_V, np.float32), np.asarray(b_O, np.float32)
    in_maps = make_in_maps(
        normalized_resid_pre, W_Q, W_K, W_V, W_O, b_Q, b_K
    )

    nc = _get_program()
    res = run_bass_kernel_spmd(nc, in_maps, list(range(8))).results

    out = np.zeros((B, S, D), np.float32)
    for c in range(8):
        out[c // 2] += np.asarray(res[c]["out"], dtype=np.float32)
    out += b_O + np.einsum("nh,nhd->d", b_V, W_O)
    return out


# revision 26
# speedup vs baseline: 1.0496x; 1.0496x over previous
"""Causal multi-head attention block on 8 NeuronCores (Trainium2, Bass/Tile).

Reference computation (per batch b):
  Q = x @ W_Q + b_Q ; K = x @ W_K + b_K ; V = x @ W_V + b_V   (per head)
  scores = Q K^T / sqrt(H); causal mask; probs = softmax(scores)
  out = (probs @ V) @ W_O + b_O

Sharding: core c -> batch c//2, head-group c%2 (6 of 12 heads).
Each core computes a partial output [S, D] (its heads' contribution,
with b_Q/b_K applied on-device). Host sums the two head-group partials
per batch and adds b_O + sum_nh b_V[n,h] * W_O[n,h,:] (exact: the b_V
term factors out because softmax rows sum to 1).

Device-side layout choices (v3, bf16 + interleaved schedule):
  - all matmul operands are bf16 (same 1 cycle/row PE rate as fp32r but
    with no >=256 moving-dim constraint); PSUM accumulation stays fp32.
    DMA traffic halves; output is written bf16 and upcast on host.
  - scores are computed transposed ([k, q]); the softmax sum over k is
    taken by the PV matmul via a ones column on V.
  - the two heads of a pair share one 2-bank PSUM score tile
    [128, 2, 512]; a single Exp activation covers both heads. Score
    tiles are double-buffered so the PE can run a k-tile ahead of Exp.
  - the attention inner loop is Activation-paced (exp ~0.9us/k-tile vs
    ~0.64us of PE work), so PE filler work (s2=1 projections, qb0
    output-projection groups) is interleaved between k-tiles to keep
    the PE busy through the attention phases.
  - qb1 of the last pair streams its softmax normalize per 256-column
    half (columns [0:256] are final after k-tile 5), so the final
    output-projection tiles overlap the tail of attention instead of
    serializing after it.
  - engine balance: exp + qb0 bias-adds + dh0 out-copies on Activation;
    reciprocal/normalize + s2=1 bias-adds + dh1 out-copies on DVE;
    causal masks, V-copies, and broadcasts on gpsimd.
  - projections, warm-up, and output-projection accumulators share one
    double-buffered 1-bank PSUM pool (8 banks total in use).
"""

import sys

sys.path.insert(0, "/opt/trn_rl_repo")

from contextlib import ExitStack

import ml_dtypes
import numpy as np

import concourse.bass as bass
import concourse.tile as tile
from concourse import bacc, mybir
from concourse.bass_utils import run_bass_kernel_spmd

B, S, D, N, H = 4, 1024, 768, 12, 64
NHC = 6            # heads per core
NPAIR = NHC // 2   # head pairs per core (2 heads stacked -> 128 partitions)
HD = NHC * H       # 384: per-core packed head dim
P = 128
NDT = D // P       # 6 d-tiles
NST = S // P       # 8 s-tiles (also k-tiles)
QB = 512           # q block (moving-dim tile for most matmuls)
NQB = S // QB      # 2
F32 = mybir.dt.float32
BF16 = mybir.dt.bfloat16
FP8 = mybir.dt.float8e4
NDC = D // 256     # 3 DoubleRow d-chunks (256 contraction rows each)
W8_SCALE = 16.0    # host-side W_Q/W_K scale keeping fp8 out of subnormals
EXP_SCALE = 1.0 / np.sqrt(float(H))

_CACHE = {}


def _build():
    nc = bacc.Bacc()
    xt8_d = nc.declare_dram_parameter("xt8", [D, S], FP8, isOutput=False)
    xtr_d = nc.declare_dram_parameter("xtr8", [D, S], FP8, isOutput=False)
    wq_d = nc.declare_dram_parameter("wq8", [D, HD], FP8, isOutput=False)
    wk_d = nc.declare_dram_parameter("wk8", [D, HD], FP8, isOutput=False)
    wv_d = nc.declare_dram_parameter("wv8", [D, HD], FP8, isOutput=False)
    wvr_d = nc.declare_dram_parameter("wvr8", [D, HD], FP8, isOutput=False)
    wo_d = nc.declare_dram_parameter("wo", [HD, D], BF16, isOutput=False)
    bq_d = nc.declare_dram_parameter("bq", [P, NPAIR], F32, isOutput=False)
    bk_d = nc.declare_dram_parameter("bk", [P, NPAIR], F32, isOutput=False)
    tri_d = nc.declare_dram_parameter("trimask", [P, 2 * P], BF16, isOutput=False)
    out_d = nc.declare_dram_parameter("out", [S, D], BF16, isOutput=True)

    # fp8 path: DoubleRow matmuls contract 2 rows per partition, so x and
    # the W matrices are addressed as [p, chunk, i, *], d = c*256 + i*128 + p.
    # xr8/wvr8 are fp8 quantization residuals: adding their cross-terms into
    # the same PSUM group (same dequant scale) cancels most of the fp8 error.
    xt8_r = xt8_d[:].rearrange("(c i p) s -> p c i s", p=P, i=2)
    xtr_r = xtr_d[:].rearrange("(c i p) s -> p c i s", p=P, i=2)
    wq_r = wq_d[:].rearrange("(c i p) h -> p c i h", p=P, i=2)
    wk_r = wk_d[:].rearrange("(c i p) h -> p c i h", p=P, i=2)
    wv_r = wv_d[:].rearrange("(c i p) h -> p c i h", p=P, i=2)
    wvr_r = wvr_d[:].rearrange("(c i p) h -> p c i h", p=P, i=2)
    wo_r = wo_d[:].rearrange("(t p) d -> p t d", p=P)

    with tile.TileContext(nc) as tc, ExitStack() as ctx:
        consts = ctx.enter_context(tc.tile_pool(name="consts", bufs=1))
        persist = ctx.enter_context(tc.tile_pool(name="persist", bufs=1))
        etp = ctx.enter_context(tc.tile_pool(name="etp", bufs=4))
        smalls = ctx.enter_context(tc.tile_pool(name="smalls", bufs=4))
        outp = ctx.enter_context(tc.tile_pool(name="outp", bufs=3))

        x8 = consts.tile([P, NDC, 2, S], FP8)
        xr8 = consts.tile([P, NDC, 2, S], FP8)
        wq_sb = consts.tile([P, NDC, 2, HD], FP8)
        wk_sb = consts.tile([P, NDC, 2, HD], FP8)
        wv_sb = consts.tile([P, NDC, 2, HD], FP8)
        wvr_sb = consts.tile([P, NDC, 2, HD], FP8)
        bq_sb = consts.tile([P, NPAIR], F32)
        bk_sb = consts.tile([P, NPAIR], F32)
        tri = consts.tile([P, 2, P], BF16)
        wo_sb = consts.tile([P, NPAIR, D], BF16)

        # ---- DMA emission order == priority order on the shared DMA device.
        # Q/K projections of all pairs run first (need only wq8/wk8 + x8 qb0),
        # then V; later phases' tensors stream behind.
        nc.sync.dma_start(out=wq_sb, in_=wq_r)
        nc.sync.dma_start(out=x8[:, :, :, 0:QB], in_=xt8_r[:, :, :, 0:QB])
        nc.sync.dma_start(out=xr8[:, :, :, 0:QB], in_=xtr_r[:, :, :, 0:QB])
        nc.sync.dma_start(out=bq_sb, in_=bq_d[:])
        nc.sync.dma_start(out=wk_sb, in_=wk_r)
        nc.sync.dma_start(out=bk_sb, in_=bk_d[:])
        nc.sync.dma_start(out=wv_sb, in_=wv_r)
        nc.sync.dma_start(out=wvr_sb, in_=wvr_r)
        nc.sync.dma_start(
            out=tri, in_=tri_d[:].rearrange("p (two q) -> p two q", two=2)
        )
        nc.sync.dma_start(out=x8[:, :, :, QB:S], in_=xt8_r[:, :, :, QB:S])
        nc.sync.dma_start(out=xr8[:, :, :, QB:S], in_=xtr_r[:, :, :, QB:S])
        nc.sync.dma_start(out=wo_sb, in_=wo_r)

        # ---- persistent activations ----
        qT = persist.tile([P, NPAIR, S], BF16)     # Q^T, head pairs stacked
        kT = persist.tile([P, NPAIR, S], BF16)
        vA = persist.tile([P, NST, NHC, H + 1], BF16)  # V + ones col, per k-tile
        zT = persist.tile([P, NPAIR, S], BF16)     # z^T (normalized), pairs stacked

        nc.gpsimd.memset(vA[:, :, :, H : H + 1], 1.0)

        # Shared 1-bank accumulator pool: warm-up, Q/K/V projections, and
        # output-projection groups all round-robin its two buffers.
        ps_big = ctx.enter_context(tc.tile_pool(name="ps_big", bufs=2, space="PSUM"))
        # Score tiles: [128, 2, 512] fp32 = 2 banks each, double-buffered.
        ps_s = ctx.enter_context(tc.tile_pool(name="ps_s", bufs=2, space="PSUM"))
        # z accumulators (one per head of the active pair): 1 bank each.
        ps_z = ctx.enter_context(tc.tile_pool(name="ps_z", bufs=1, space="PSUM"))

        # PE warm-up: matmuls on a zeroed tile depend on no DMA, so they run
        # during the input-stream prologue and carry the PE clock (HAM) and
        # cost-model p-state ramp to full speed before the first real matmul.
        dums = consts.tile([P, QB], BF16)
        nc.gpsimd.memset(dums, 0.0)
        # Activation-table preload: the first table-based activation pays a
        # 1283ns ACT_TABLE_LOAD; trigger it at t=0 on a dummy tile so the
        # first bias-add (which gates the ps_big ring) doesn't.
        actwarm = consts.tile([1, 1], F32)
        nc.gpsimd.memset(actwarm, 0.0)
        nc.scalar.activation(
            actwarm, actwarm, mybir.ActivationFunctionType.Exp
        )
        wps = ps_big.tile([P, QB], F32, name="warm", tag="big")
        for i in range(12):
            nc.tensor.matmul(
                wps,
                dums[:, 0:P],
                dums,
                start=(i == 0),
                stop=(i == 11),
            )

        def proj_one(w_sb, b_sb, dst, g, s2, eng):
            """Q/K projection of one head pair over one q-half: fp8 DoubleRow
            matmuls (256 contraction rows per instruction, 0.5 cycles/row),
            two sets (x8 + its fp8 residual) accumulated in one PSUM group.
            The bias-add applies the 1/W8_SCALE dequant."""
            ps = ps_big.tile([P, QB], F32, tag="big")
            for si, xs in enumerate((x8, xr8)):
                for c in range(NDC):
                    nc.tensor.matmul(
                        ps,
                        w_sb[:, c, :, g * P : (g + 1) * P],
                        xs[:, c, :, s2 * QB : (s2 + 1) * QB],
                        start=(si == 0 and c == 0),
                        stop=(si == 1 and c == NDC - 1),
                        perf_mode=mybir.MatmulPerfMode.DoubleRow,
                    )
            dst_ap = dst[:, g, s2 * QB : (s2 + 1) * QB]
            if eng == "act":
                nc.scalar.activation(
                    dst_ap,
                    ps,
                    mybir.ActivationFunctionType.Identity,
                    bias=b_sb[:, g : g + 1],
                    scale=1.0 / W8_SCALE,
                )
            else:
                nc.vector.tensor_scalar(
                    dst_ap,
                    ps,
                    1.0 / W8_SCALE,
                    b_sb[:, g : g + 1],
                    mybir.AluOpType.mult,
                    mybir.AluOpType.add,
                )

        def proj_v(st):
            """V projection of one k-tile: three fp8 DoubleRow sets
            (x8*wv8 + xr8*wv8 + x8*wvr8 — both residual cross-terms, same
            dequant scale) so V carries ~0.2% error despite fp8 operands."""
            vps = ps_big.tile([P, HD], F32, tag="big")
            sets = ((x8, wv_sb), (xr8, wv_sb), (x8, wvr_sb))
            for si, (xs, ws) in enumerate(sets):
                for c in range(NDC):
                    nc.tensor.matmul(
                        vps,
                        xs[:, c, :, st * P : (st + 1) * P],
                        ws[:, c, :, :],
                        start=(si == 0 and c == 0),
                        stop=(si == 2 and c == NDC - 1),
                        perf_mode=mybir.MatmulPerfMode.DoubleRow,
                    )
            nc.vector.tensor_scalar_mul(
                vA[:, st, :, 0:H],
                vps.rearrange("p (n h) -> p n h", n=NHC),
                1.0 / W8_SCALE,
            )

        def norm_block(zzps, g, q0, c0, c1):
            """Normalize z columns [c0, c1) of pair g's block at q offset q0."""
            w = c1 - c0
            for hh in range(2):
                hp = hh * H
                r = smalls.tile([1, w], F32, tag="r")
                nc.vector.reciprocal(r, zzps[hh][H : H + 1, c0:c1])
                rb = smalls.tile([H, w], F32, tag="rb")
                nc.gpsimd.partition_broadcast(rb, r)
                nc.vector.tensor_mul(
                    zT[hp : hp + H, g, q0 + c0 : q0 + c1],
                    zzps[hh][0:H, c0:c1],
                    rb,
                )

        def attend_pair(g, qb, fillers=None, post_kt=None):
            """Both heads of pair g: one 2-bank score tile per k-tile, one
            merged Exp per k-tile; PV accumulates z^T + softmax denominator
            via the ones column of vA. `fillers` are PE work units popped
            one per k-tile to cover the Exp-paced stretches; `post_kt` maps
            k-tile index -> closures run right after that k-tile's PV (used
            to stream the last pair's normalize + output tiles)."""
            q0 = qb * QB
            nkt = (qb + 1) * QB // P  # causal: k-tiles 0..nkt-1
            zzps = [
                ps_z.tile([H + 1, QB], F32, name=f"zps{hh}", tag=f"z{hh}")
                for hh in range(2)
            ]
            # One filler ahead of k-tile 0: its PV may wait on the previous
            # pair's normalize to release the z banks.
            if fillers:
                fillers.pop(0)()
            for kt in range(nkt):
                o = max(kt * P - q0, 0)  # first live column
                sps = ps_s.tile([P, 2, QB], F32, tag="s")
                for hh in range(2):
                    hp = hh * H
                    nc.tensor.matmul(
                        sps[:, hh, o:QB],
                        kT[hp : hp + H, g, kt * P : (kt + 1) * P],
                        qT[hp : hp + H, g, q0 + o : q0 + QB],
                        start=True,
                        stop=True,
                        tile_position=(hp, 0),
                    )
                et = etp.tile([P, 2, QB], BF16)
                nc.scalar.activation(
                    et[:, :, o:QB],
                    sps[:, :, o:QB],
                    mybir.ActivationFunctionType.Exp,
                    scale=EXP_SCALE,
                )
                if kt * P - q0 >= -(P - 1):  # diagonal tile: partial block
                    nc.gpsimd.tensor_mul(
                        et[:, :, o : o + P], et[:, :, o : o + P], tri
                    )
                for hh in range(2):
                    nc.tensor.matmul(
                        zzps[hh][:, o:QB],
                        vA[:, kt, 2 * g + hh, :],
                        et[:, hh, o:QB],
                        start=(kt == 0),
                        stop=(kt == nkt - 1),
                    )
                if post_kt and kt in post_kt:
                    for fn in post_kt[kt]:
                        fn(zzps)
                if fillers:
                    fillers.pop(0)()
            return zzps

        def out_group(row0, dh, out_t, copy_eng="dve", ops=None):
            """One output-projection accumulation group: rows [row0, row0+P),
            column half dh."""
            if ops is None:
                ops = ps_big.tile([P, D // 2], F32, tag="big")
            for g in range(NPAIR):
                nc.tensor.matmul(
                    ops,
                    zT[:, g, row0 : row0 + P],
                    wo_sb[:, g, dh * (D // 2) : (dh + 1) * (D // 2)],
                    start=(g == 0),
                    stop=(g == NPAIR - 1),
                )
            dst = out_t[:, dh * (D // 2) : (dh + 1) * (D // 2)]
            if copy_eng == "act":
                nc.scalar.copy(dst, ops)
            else:
                nc.vector.tensor_copy(out=dst, in_=ops)

        def out_tile(row0, tail=False):
            """Full output tile rows [row0, row0+P): both dh groups, one DMA.
            Tiles emitted after the last Exp borrow the (drained) score pool
            for their accumulators and split copies across Act/DVE, so the
            serial tail is just mm -> copy -> DMA."""
            out_t = outp.tile([P, D], BF16)
            if tail:
                ops2 = ps_s.tile([P, 2, QB], F32, tag="s")
                out_group(row0, 0, out_t, copy_eng="act", ops=ops2[:, 0, 0 : D // 2])
                out_group(row0, 1, out_t, ops=ops2[:, 1, 0 : D // 2])
            else:
                out_group(row0, 0, out_t)
                out_group(row0, 1, out_t)
            nc.sync.dma_start(out=out_d[row0 : row0 + P, :], in_=out_t)

        # ---- schedule: all qb0 Q/K projections first (cheap fp8 DoubleRow,
        # need only wq8/wk8 + x8 qb0); V projections and qb0 output tiles
        # ride as fillers inside the Activation-paced attention loops so the
        # PE never idles on Exp.
        for g in range(NPAIR):
            proj_one(wq_sb, bq_sb, qT, g, 0, "act")
        proj_one(wk_sb, bk_sb, kT, 0, 0, "act")
        proj_one(wk_sb, bk_sb, kT, 1, 0, "dve")
        proj_one(wk_sb, bk_sb, kT, 2, 0, "dve")
        proj_v(0)
        z = attend_pair(0, 0, fillers=[
            lambda: proj_v(1),
            lambda: proj_v(2),
            lambda: proj_v(3),
        ])
        norm_block(z, 0, 0, 0, QB)
        z = attend_pair(1, 0, fillers=[
            lambda: proj_one(wq_sb, bq_sb, qT, 0, 1, "dve"),
            lambda: proj_one(wk_sb, bk_sb, kT, 0, 1, "dve"),
        ])
        norm_block(z, 1, 0, 0, QB)
        z = attend_pair(2, 0, fillers=[
            lambda: proj_v(4),
            lambda: proj_v(5),
            lambda: proj_v(6),
            lambda: proj_v(7),
        ])
        norm_block(z, 2, 0, 0, QB)

        z = attend_pair(0, 1, fillers=[
            lambda: proj_one(wq_sb, bq_sb, qT, 1, 1, "dve"),
            lambda: proj_one(wk_sb, bk_sb, kT, 1, 1, "dve"),
            lambda: out_tile(0 * P),
            lambda: proj_one(wq_sb, bq_sb, qT, 2, 1, "dve"),
            lambda: proj_one(wk_sb, bk_sb, kT, 2, 1, "dve"),
            lambda: out_tile(1 * P),
        ])
        norm_block(z, 0, QB, 0, QB)

        f11 = [lambda qt=qt: out_tile(qt * P) for qt in range(2, 4)]
        z = attend_pair(1, 1, fillers=f11)
        norm_block(z, 1, QB, 0, QB)

        # Last pair: stream the normalize per 128-column block (block qt of
        # qb1 is final after k-tile 4+qt), so qb1 output tiles overlap the
        # tail of attention; norms are emitted before out-tiles so the DVE
        # starts each block's chain while the PE runs the out-tile matmuls.
        post = {
            4: [lambda zz: norm_block(zz, 2, QB, 0, P)],
            5: [
                lambda zz: norm_block(zz, 2, QB, P, 2 * P),
                lambda zz: out_tile(QB + 0 * P),
            ],
            6: [
                lambda zz: norm_block(zz, 2, QB, 2 * P, 3 * P),
                lambda zz: out_tile(QB + 1 * P),
            ],
            7: [
                lambda zz: norm_block(zz, 2, QB, 3 * P, QB),
                lambda zz: out_tile(QB + 2 * P, tail=True),
            ],
        }
        z = attend_pair(2, 1, post_kt=post)
        out_tile(QB + 3 * P, tail=True)

    if not nc.is_finalized():
        nc.finalize()
    return nc


def _get_program():
    if "nc" not in _CACHE:
        _CACHE["nc"] = _build()
    return _CACHE["nc"]


def make_in_maps(
    normalized_resid_pre, W_Q, W_K, W_V, W_O, b_Q, b_K, b_V=None, b_O=None, **_unused
):
    bf = ml_dtypes.bfloat16
    f8 = ml_dtypes.float8_e4m3
    x = np.asarray(normalized_resid_pre, np.float32)
    W_Q, W_K, W_V = (np.asarray(a, np.float32) for a in (W_Q, W_K, W_V))
    W_O = np.asarray(W_O, np.float32)
    b_Q, b_K = np.asarray(b_Q, np.float32), np.asarray(b_K, np.float32)

    tri = np.triu(np.ones((P, P), np.float32))
    tri2 = np.concatenate([tri, tri], axis=1).astype(bf)
    in_maps = []
    for c in range(8):
        b, hg = divmod(c, 2)
        hs = slice(hg * NHC, (hg + 1) * NHC)
        xt = x[b].T
        xt8 = xt.astype(f8)
        wq8 = (W_Q[hs] * W8_SCALE).transpose(1, 0, 2).reshape(D, HD).astype(f8)
        wk8 = (W_K[hs] * W8_SCALE).transpose(1, 0, 2).reshape(D, HD).astype(f8)
        wvs = (W_V[hs] * W8_SCALE).transpose(1, 0, 2).reshape(D, HD)
        wv8 = wvs.astype(f8)
        in_maps.append(
            {
                "xt8": np.ascontiguousarray(xt8),
                "xtr8": np.ascontiguousarray(
                    (xt - xt8.astype(np.float32)).astype(f8)
                ),
                "wq8": np.ascontiguousarray(wq8),
                "wk8": np.ascontiguousarray(wk8),
                "wv8": np.ascontiguousarray(wv8),
                "wvr8": np.ascontiguousarray(
                    (wvs - wv8.astype(np.float32)).astype(f8)
                ),
                "wo": np.ascontiguousarray(W_O[hs].reshape(HD, D).astype(bf)),
                "bq": np.ascontiguousarray(b_Q[hs].reshape(NPAIR, P).T),
                "bk": np.ascontiguousarray(b_K[hs].reshape(NPAIR, P).T),
                "trimask": tri2,
            }
        )
    return in_maps


def kernel(
    normalized_resid_pre, W_Q, W_K, W_V, W_O, b_Q, b_K, b_V, b_O, **_unused
):
    W_O = np.asarray(W_O, np.float32)
    b_V, b_O = np.asarray(b_V, np.float32), np.asarray(b_O, np.float32)
    in_maps = make_in_maps(
        normalized_resid_pre, W_Q, W_K, W_V, W_O, b_Q, b_K
    )

    nc = _get_program()
    res = run_bass_kernel_spmd(nc, in_maps, list(range(8))).results

    out = np.zeros((B, S, D), np.float32)
    for c in range(8):
        out[c // 2] += np.asarray(res[c]["out"], dtype=np.float32)
    out += b_O + np.einsum("nh,nhd->d", b_V, W_O)
    return out


# revision 27
# speedup vs baseline: 1.0707x; 1.0201x over previous
"""Causal multi-head attention block on 8 NeuronCores (Trainium2, Bass/Tile).

Reference computation (per batch b):
  Q = x @ W_Q + b_Q ; K = x @ W_K + b_K ; V = x @ W_V + b_V   (per head)
  scores = Q K^T / sqrt(H); causal mask; probs = softmax(scores)
  out = (probs @ V) @ W_O + b_O

Sharding: core c -> batch c//2, head-group c%2 (6 of 12 heads).
Each core computes a partial output [S, D] (its heads' contribution,
with b_Q/b_K applied on-device). Host sums the two head-group partials
per batch and adds b_O + sum_nh b_V[n,h] * W_O[n,h,:] (exact: the b_V
term factors out because softmax rows sum to 1).

Device-side layout choices (v3, bf16 + interleaved schedule):
  - all matmul operands are bf16 (same 1 cycle/row PE rate as fp32r but
    with no >=256 moving-dim constraint); PSUM accumulation stays fp32.
    DMA traffic halves; output is written bf16 and upcast on host.
  - scores are computed transposed ([k, q]); the softmax sum over k is
    taken by the PV matmul via a ones column on V.
  - the two heads of a pair share one 2-bank PSUM score tile
    [128, 2, 512]; a single Exp activation covers both heads. Score
    tiles are double-buffered so the PE can run a k-tile ahead of Exp.
  - the attention inner loop is Activation-paced (exp ~0.9us/k-tile vs
    ~0.64us of PE work), so PE filler work (s2=1 projections, qb0
    output-projection groups) is interleaved between k-tiles to keep
    the PE busy through the attention phases.
  - qb1 of the last pair streams its softmax normalize per 256-column
    half (columns [0:256] are final after k-tile 5), so the final
    output-projection tiles overlap the tail of attention instead of
    serializing after it.
  - engine balance: exp + qb0 bias-adds + dh0 out-copies on Activation;
    reciprocal/normalize + s2=1 bias-adds + dh1 out-copies on DVE;
    causal masks, V-copies, and broadcasts on gpsimd.
  - projections, warm-up, and output-projection accumulators share one
    double-buffered 1-bank PSUM pool (8 banks total in use).
"""

import sys

sys.path.insert(0, "/opt/trn_rl_repo")

from contextlib import ExitStack

import ml_dtypes
import numpy as np

import concourse.bass as bass
import concourse.tile as tile
from concourse import bacc, mybir
from concourse.bass_utils import run_bass_kernel_spmd

B, S, D, N, H = 4, 1024, 768, 12, 64
NHC = 6            # heads per core
NPAIR = NHC // 2   # head pairs per core (2 heads stacked -> 128 partitions)
HD = NHC * H       # 384: per-core packed head dim
P = 128
NDT = D // P       # 6 d-tiles
NST = S // P       # 8 s-tiles (also k-tiles)
QB = 512           # q block (moving-dim tile for most matmuls)
NQB = S // QB      # 2
F32 = mybir.dt.float32
BF16 = mybir.dt.bfloat16
FP8 = mybir.dt.float8e4
NDC = D // 256     # 3 DoubleRow d-chunks (256 contraction rows each)
W8_SCALE = 16.0    # host-side W_Q/W_K scale keeping fp8 out of subnormals
EXP_SCALE = 1.0 / np.sqrt(float(H))

_CACHE = {}


def _build():
    nc = bacc.Bacc()
    xt8_d = nc.declare_dram_parameter("xt8", [D, S], FP8, isOutput=False)
    xtr_d = nc.declare_dram_parameter("xtr8", [D, S], FP8, isOutput=False)
    wq_d = nc.declare_dram_parameter("wq8", [D, HD], FP8, isOutput=False)
    wk_d = nc.declare_dram_parameter("wk8", [D, HD], FP8, isOutput=False)
    wv_d = nc.declare_dram_parameter("wv8", [D, HD], FP8, isOutput=False)
    wvr_d = nc.declare_dram_parameter("wvr8", [D, HD], FP8, isOutput=False)
    wo_d = nc.declare_dram_parameter("wo", [HD, D], BF16, isOutput=False)
    bq_d = nc.declare_dram_parameter("bq", [P, NPAIR], F32, isOutput=False)
    bk_d = nc.declare_dram_parameter("bk", [P, NPAIR], F32, isOutput=False)
    tri_d = nc.declare_dram_parameter("trimask", [P, 2 * P], BF16, isOutput=False)
    out_d = nc.declare_dram_parameter("out", [S, D], BF16, isOutput=True)

    # fp8 path: DoubleRow matmuls contract 2 rows per partition, so x and
    # the W matrices are addressed as [p, chunk, i, *], d = c*256 + i*128 + p.
    # xr8/wvr8 are fp8 quantization residuals: adding their cross-terms into
    # the same PSUM group (same dequant scale) cancels most of the fp8 error.
    xt8_r = xt8_d[:].rearrange("(c i p) s -> p c i s", p=P, i=2)
    xtr_r = xtr_d[:].rearrange("(c i p) s -> p c i s", p=P, i=2)
    wq_r = wq_d[:].rearrange("(c i p) h -> p c i h", p=P, i=2)
    wk_r = wk_d[:].rearrange("(c i p) h -> p c i h", p=P, i=2)
    wv_r = wv_d[:].rearrange("(c i p) h -> p c i h", p=P, i=2)
    wvr_r = wvr_d[:].rearrange("(c i p) h -> p c i h", p=P, i=2)
    wo_r = wo_d[:].rearrange("(t p) d -> p t d", p=P)

    with tile.TileContext(nc) as tc, ExitStack() as ctx:
        consts = ctx.enter_context(tc.tile_pool(name="consts", bufs=1))
        persist = ctx.enter_context(tc.tile_pool(name="persist", bufs=1))
        etp = ctx.enter_context(tc.tile_pool(name="etp", bufs=4))
        smalls = ctx.enter_context(tc.tile_pool(name="smalls", bufs=4))
        outp = ctx.enter_context(tc.tile_pool(name="outp", bufs=3))

        x8 = consts.tile([P, NDC, 2, S], FP8)
        xr8 = consts.tile([P, NDC, 2, S], FP8)
        wq_sb = consts.tile([P, NDC, 2, HD], FP8)
        wk_sb = consts.tile([P, NDC, 2, HD], FP8)
        wv_sb = consts.tile([P, NDC, 2, HD], FP8)
        wvr_sb = consts.tile([P, NDC, 2, HD], FP8)
        bq_sb = consts.tile([P, NPAIR], F32)
        bk_sb = consts.tile([P, NPAIR], F32)
        tri = consts.tile([P, 2, P], BF16)
        wo_sb = consts.tile([P, NPAIR, D], BF16)

        # ---- DMA emission order == priority order on the shared DMA device.
        # Q/K projections of all pairs run first (need only wq8/wk8 + x8 qb0),
        # then V; later phases' tensors stream behind.
        nc.sync.dma_start(out=wq_sb, in_=wq_r)
        nc.sync.dma_start(out=x8[:, :, :, 0:QB], in_=xt8_r[:, :, :, 0:QB])
        nc.sync.dma_start(out=xr8[:, :, :, 0:QB], in_=xtr_r[:, :, :, 0:QB])
        nc.sync.dma_start(out=bq_sb, in_=bq_d[:])
        nc.sync.dma_start(out=wk_sb, in_=wk_r)
        nc.sync.dma_start(out=bk_sb, in_=bk_d[:])
        nc.sync.dma_start(out=wv_sb, in_=wv_r)
        nc.sync.dma_start(out=wvr_sb, in_=wvr_r)
        nc.sync.dma_start(
            out=tri, in_=tri_d[:].rearrange("p (two q) -> p two q", two=2)
        )
        nc.sync.dma_start(out=x8[:, :, :, QB:S], in_=xt8_r[:, :, :, QB:S])
        nc.sync.dma_start(out=xr8[:, :, :, QB:S], in_=xtr_r[:, :, :, QB:S])
        nc.sync.dma_start(out=wo_sb, in_=wo_r)

        # ---- persistent activations ----
        qT = persist.tile([P, NPAIR, S], BF16)     # Q^T, head pairs stacked
        kT = persist.tile([P, NPAIR, S], BF16)
        vA = persist.tile([P, NST, NHC, H + 1], BF16)  # V + ones col, per k-tile
        zT = persist.tile([P, NPAIR, S], BF16)     # z^T (normalized), pairs stacked

        nc.gpsimd.memset(vA[:, :, :, H : H + 1], 1.0)

        # Shared 1-bank accumulator pool: warm-up, Q/K/V projections, and
        # output-projection groups all round-robin its two buffers.
        ps_big = ctx.enter_context(tc.tile_pool(name="ps_big", bufs=2, space="PSUM"))
        # Score tiles: [128, 2, 512] fp32 = 2 banks each, double-buffered.
        ps_s = ctx.enter_context(tc.tile_pool(name="ps_s", bufs=2, space="PSUM"))
        # z accumulators (one per head of the active pair): 1 bank each.
        ps_z = ctx.enter_context(tc.tile_pool(name="ps_z", bufs=1, space="PSUM"))

        # PE warm-up: matmuls on a zeroed tile depend on no DMA, so they run
        # during the input-stream prologue and carry the PE clock (HAM) and
        # cost-model p-state ramp to full speed before the first real matmul.
        dums = consts.tile([P, QB], BF16)
        nc.gpsimd.memset(dums, 0.0)
        # Activation-table preload: the first table-based activation pays a
        # 1283ns ACT_TABLE_LOAD; trigger it at t=0 on a dummy tile so the
        # first bias-add (which gates the ps_big ring) doesn't.
        actwarm = consts.tile([1, 1], F32)
        nc.gpsimd.memset(actwarm, 0.0)
        nc.scalar.activation(
            actwarm, actwarm, mybir.ActivationFunctionType.Exp
        )
        wps = ps_big.tile([P, QB], F32, name="warm", tag="big")
        for i in range(12):
            nc.tensor.matmul(
                wps,
                dums[:, 0:P],
                dums,
                start=(i == 0),
                stop=(i == 11),
            )

        def proj_one(w_sb, b_sb, dst, g, s2, eng):
            """Q/K projection of one head pair over one q-half: fp8 DoubleRow
            matmuls (256 contraction rows per instruction, 0.5 cycles/row),
            two sets (x8 + its fp8 residual) accumulated in one PSUM group.
            The bias-add applies the 1/W8_SCALE dequant."""
            ps = ps_big.tile([P, QB], F32, tag="big")
            for si, xs in enumerate((x8, xr8)):
                for c in range(NDC):
                    nc.tensor.matmul(
                        ps,
                        w_sb[:, c, :, g * P : (g + 1) * P],
                        xs[:, c, :, s2 * QB : (s2 + 1) * QB],
                        start=(si == 0 and c == 0),
                        stop=(si == 1 and c == NDC - 1),
                        perf_mode=mybir.MatmulPerfMode.DoubleRow,
                    )
            dst_ap = dst[:, g, s2 * QB : (s2 + 1) * QB]
            if eng == "act":
                nc.scalar.activation(
                    dst_ap,
                    ps,
                    mybir.ActivationFunctionType.Identity,
                    bias=b_sb[:, g : g + 1],
                    scale=1.0 / W8_SCALE,
                )
            else:
                nc.vector.tensor_scalar(
                    dst_ap,
                    ps,
                    1.0 / W8_SCALE,
                    b_sb[:, g : g + 1],
                    mybir.AluOpType.mult,
                    mybir.AluOpType.add,
                )

        def proj_v(st):
            """V projection of one k-tile: three fp8 DoubleRow sets
            (x8*wv8 + xr8*wv8 + x8*wvr8 — both residual cross-terms, same
            dequant scale) so V carries ~0.2% error despite fp8 operands."""
            vps = ps_big.tile([P, HD], F32, tag="big")
            sets = ((x8, wv_sb), (xr8, wv_sb), (x8, wvr_sb))
            for si, (xs, ws) in enumerate(sets):
                for c in range(NDC):
                    nc.tensor.matmul(
                        vps,
                        xs[:, c, :, st * P : (st + 1) * P],
                        ws[:, c, :, :],
                        start=(si == 0 and c == 0),
                        stop=(si == 2 and c == NDC - 1),
                        perf_mode=mybir.MatmulPerfMode.DoubleRow,
                    )
            nc.vector.tensor_scalar_mul(
                vA[:, st, :, 0:H],
                vps.rearrange("p (n h) -> p n h", n=NHC),
                1.0 / W8_SCALE,
            )

        def norm_block(zzps, g, q0, c0, c1):
            """Normalize z columns [c0, c1) of pair g's block at q offset q0."""
            w = c1 - c0
            for hh in range(2):
                hp = hh * H
                r = smalls.tile([1, w], F32, tag="r")
                nc.vector.reciprocal(r, zzps[hh][H : H + 1, c0:c1])
                rb = smalls.tile([H, w], F32, tag="rb")
                nc.gpsimd.partition_broadcast(rb, r)
                nc.vector.tensor_mul(
                    zT[hp : hp + H, g, q0 + c0 : q0 + c1],
                    zzps[hh][0:H, c0:c1],
                    rb,
                )

        def attend_pair(g, qb, fillers=None, post_kt=None):
            """Both heads of pair g: one 2-bank score tile per k-tile, one
            merged Exp per k-tile; PV accumulates z^T + softmax denominator
            via the ones column of vA. `fillers` are PE work units popped
            one per k-tile to cover the Exp-paced stretches; `post_kt` maps
            k-tile index -> closures run right after that k-tile's PV (used
            to stream the last pair's normalize + output tiles)."""
            q0 = qb * QB
            nkt = (qb + 1) * QB // P  # causal: k-tiles 0..nkt-1
            zzps = [
                ps_z.tile([H + 1, QB], F32, name=f"zps{hh}", tag=f"z{hh}")
                for hh in range(2)
            ]
            # One filler ahead of k-tile 0: its PV may wait on the previous
            # pair's normalize to release the z banks.
            if fillers:
                fillers.pop(0)()

            def scores(kt):
                """Score matmuls + merged Exp + causal mask for one k-tile."""
                o = max(kt * P - q0, 0)  # first live column
                sps = ps_s.tile([P, 2, QB], F32, tag="s")
                for hh in range(2):
                    hp = hh * H
                    nc.tensor.matmul(
                        sps[:, hh, o:QB],
                        kT[hp : hp + H, g, kt * P : (kt + 1) * P],
                        qT[hp : hp + H, g, q0 + o : q0 + QB],
                        start=True,
                        stop=True,
                        tile_position=(hp, 0),
                    )
                et = etp.tile([P, 2, QB], BF16)
                nc.scalar.activation(
                    et[:, :, o:QB],
                    sps[:, :, o:QB],
                    mybir.ActivationFunctionType.Exp,
                    scale=EXP_SCALE,
                )
                if kt * P - q0 >= -(P - 1):  # diagonal tile: partial block
                    # final k-tile's mask gates the pair-end chain: run it on
                    # DVE (fast bf16 path) instead of gpsimd
                    eng = nc.vector if kt == nkt - 1 else nc.gpsimd
                    eng.tensor_mul(
                        et[:, :, o : o + P], et[:, :, o : o + P], tri
                    )
                return et, o

            # Software-pipeline by one k-tile: the PE is in-order, so the
            # next tile's (exp-independent) score matmuls are emitted before
            # this tile's PV, which must wait on the Exp.
            pending = scores(0)
            for kt in range(nkt):
                if kt + 1 < nkt:
                    nxt = scores(kt + 1)
                et, o = pending
                for hh in range(2):
                    nc.tensor.matmul(
                        zzps[hh][:, o:QB],
                        vA[:, kt, 2 * g + hh, :],
                        et[:, hh, o:QB],
                        start=(kt == 0),
                        stop=(kt == nkt - 1),
                    )
                if kt + 1 < nkt:
                    pending = nxt
                if post_kt and kt in post_kt:
                    for fn in post_kt[kt]:
                        fn(zzps)
                if fillers:
                    fillers.pop(0)()
            return zzps

        def out_group(row0, dh, out_t, copy_eng="dve", ops=None):
            """One output-projection accumulation group: rows [row0, row0+P),
            column half dh."""
            if ops is None:
                ops = ps_big.tile([P, D // 2], F32, tag="big")
            for g in range(NPAIR):
                nc.tensor.matmul(
                    ops,
                    zT[:, g, row0 : row0 + P],
                    wo_sb[:, g, dh * (D // 2) : (dh + 1) * (D // 2)],
                    start=(g == 0),
                    stop=(g == NPAIR - 1),
                )
            dst = out_t[:, dh * (D // 2) : (dh + 1) * (D // 2)]
            if copy_eng == "act":
                nc.scalar.copy(dst, ops)
            else:
                nc.vector.tensor_copy(out=dst, in_=ops)

        def out_tile(row0, tail=False):
            """Full output tile rows [row0, row0+P): both dh groups, one DMA.
            Tiles emitted after the last Exp borrow the (drained) score pool
            for their accumulators and split copies across Act/DVE, so the
            serial tail is just mm -> copy -> DMA."""
            out_t = outp.tile([P, D], BF16)
            if tail:
                ops2 = ps_s.tile([P, 2, QB], F32, tag="s")
                out_group(row0, 0, out_t, copy_eng="act", ops=ops2[:, 0, 0 : D // 2])
                out_group(row0, 1, out_t, ops=ops2[:, 1, 0 : D // 2])
            else:
                out_group(row0, 0, out_t)
                out_group(row0, 1, out_t)
            nc.sync.dma_start(out=out_d[row0 : row0 + P, :], in_=out_t)

        # ---- schedule: all qb0 Q/K projections first (cheap fp8 DoubleRow,
        # need only wq8/wk8 + x8 qb0); V projections and qb0 output tiles
        # ride as fillers inside the Activation-paced attention loops so the
        # PE never idles on Exp.
        for g in range(NPAIR):
            proj_one(wq_sb, bq_sb, qT, g, 0, "act")
        proj_one(wk_sb, bk_sb, kT, 0, 0, "act")
        proj_one(wk_sb, bk_sb, kT, 1, 0, "dve")
        proj_one(wk_sb, bk_sb, kT, 2, 0, "dve")
        proj_v(0)
        z = attend_pair(0, 0, fillers=[
            lambda: proj_v(1),
            lambda: proj_v(2),
            lambda: proj_v(3),
        ])
        norm_block(z, 0, 0, 0, QB)
        z = attend_pair(1, 0, fillers=[
            lambda: proj_one(wq_sb, bq_sb, qT, 0, 1, "dve"),
            lambda: proj_one(wk_sb, bk_sb, kT, 0, 1, "dve"),
        ])
        norm_block(z, 1, 0, 0, QB)
        z = attend_pair(2, 0, fillers=[
            lambda: proj_v(4),
            lambda: proj_v(5),
            lambda: proj_v(6),
            lambda: proj_v(7),
        ])
        norm_block(z, 2, 0, 0, QB)

        z = attend_pair(0, 1, fillers=[
            lambda: proj_one(wq_sb, bq_sb, qT, 1, 1, "dve"),
            lambda: proj_one(wk_sb, bk_sb, kT, 1, 1, "dve"),
            lambda: out_tile(0 * P),
            lambda: proj_one(wq_sb, bq_sb, qT, 2, 1, "dve"),
            lambda: proj_one(wk_sb, bk_sb, kT, 2, 1, "dve"),
            lambda: out_tile(1 * P),
        ])
        norm_block(z, 0, QB, 0, QB)

        f11 = [lambda qt=qt: out_tile(qt * P) for qt in range(2, 4)]
        z = attend_pair(1, 1, fillers=f11)
        norm_block(z, 1, QB, 0, QB)

        # Last pair: stream the normalize per 128-column block (block qt of
        # qb1 is final after k-tile 4+qt), so qb1 output tiles overlap the
        # tail of attention; norms are emitted before out-tiles so the DVE
        # starts each block's chain while the PE runs the out-tile matmuls.
        post = {
            4: [lambda zz: norm_block(zz, 2, QB, 0, P)],
            5: [
                lambda zz: norm_block(zz, 2, QB, P, 2 * P),
                lambda zz: out_tile(QB + 0 * P),
            ],
            6: [
                lambda zz: norm_block(zz, 2, QB, 2 * P, 3 * P),
                lambda zz: out_tile(QB + 1 * P),
            ],
            7: [
                lambda zz: norm_block(zz, 2, QB, 3 * P, QB),
                lambda zz: out_tile(QB + 2 * P, tail=True),
            ],
        }
        z = attend_pair(2, 1, post_kt=post)
        out_tile(QB + 3 * P, tail=True)

    if not nc.is_finalized():
        nc.finalize()
    return nc


def _get_program():
    if "nc" not in _CACHE:
        _CACHE["nc"] = _build()
    return _CACHE["nc"]


def make_in_maps(
    normalized_resid_pre, W_Q, W_K, W_V, W_O, b_Q, b_K, b_V=None, b_O=None, **_unused
):
    bf = ml_dtypes.bfloat16
    f8 = ml_dtypes.float8_e4m3
    x = np.asarray(normalized_resid_pre, np.float32)
    W_Q, W_K, W_V = (np.asarray(a, np.float32) for a in (W_Q, W_K, W_V))
    W_O = np.asarray(W_O, np.float32)
    b_Q, b_K = np.asarray(b_Q, np.float32), np.asarray(b_K, np.float32)

    tri = np.triu(np.ones((P, P), np.float32))
    tri2 = np.concatenate([tri, tri], axis=1).astype(bf)
    in_maps = []
    for c in range(8):
        b, hg = divmod(c, 2)
        hs = slice(hg * NHC, (hg + 1) * NHC)
        xt = x[b].T
        xt8 = xt.astype(f8)
        wq8 = (W_Q[hs] * W8_SCALE).transpose(1, 0, 2).reshape(D, HD).astype(f8)
        wk8 = (W_K[hs] * W8_SCALE).transpose(1, 0, 2).reshape(D, HD).astype(f8)
        wvs = (W_V[hs] * W8_SCALE).transpose(1, 0, 2).reshape(D, HD)
        wv8 = wvs.astype(f8)
        in_maps.append(
            {
                "xt8": np.ascontiguousarray(xt8),
                "xtr8": np.ascontiguousarray(
                    (xt - xt8.astype(np.float32)).astype(f8)
                ),
                "wq8": np.ascontiguousarray(wq8),
                "wk8": np.ascontiguousarray(wk8),
                "wv8": np.ascontiguousarray(wv8),
                "wvr8": np.ascontiguousarray(
                    (wvs - wv8.astype(np.float32)).astype(f8)
                ),
                "wo": np.ascontiguousarray(W_O[hs].reshape(HD, D).astype(bf)),
                "bq": np.ascontiguousarray(b_Q[hs].reshape(NPAIR, P).T),
                "bk": np.ascontiguousarray(b_K[hs].reshape(NPAIR, P).T),
                "trimask": tri2,
            }
        )
    return in_maps


def kernel(
    normalized_resid_pre, W_Q, W_K, W_V, W_O, b_Q, b_K, b_V, b_O, **_unused
):
    W_O = np.asarray(W_O, np.float32)
    b_V, b_O = np.asarray(b_V, np.float32), np.asarray(b_O, np.float32)
    in_maps = make_in_maps(
        normalized_resid_pre, W_Q, W_K, W_V, W_O, b_Q, b_K
    )

    nc = _get_program()
    res = run_bass_kernel_spmd(nc, in_maps, list(range(8))).results

    out = np.zeros((B, S, D), np.float32)
    for c in range(8):
        out[c // 2] += np.asarray(res[c]["out"], dtype=np.float32)
    out += b_O + np.einsum("nh,nhd->d", b_V, W_O)
    return out
